# revision 21
# baseline (speedup 1.0000x reference)
"""Bidirectional cross-attention Trainium2 kernel (G-folded projections).

Data-parallel over batch B=8 across 8 NeuronCores (1 sample/core).

Logits are computed via the associativity fold
  S1 = Q1^T K2 = x1^T (Wq1^T Wk2) x2 = x1^T T2,   T2 = G1 x2
with G1 = Wq1^T Wk2 precomputed on host — this removes BOTH the Q and K
projections (one T-projection remains per direction) and lets raw x1 act
as the moving operand of the logit matmuls. The bias cross-terms
decompose into a per-i shift (cancels in softmax -> dropped exactly) and
a per-j shift psi_j = bq1.(Wk2 x2_j + bk2), computed on host (O(CN)) and
applied for free through the per-partition bias port of the ScalarE exp.

V path: VT[n,c] bf16 with NO bias (b_v commutes through softmax since
weights sum to 1 after the division; host folds it into the residual
stream xtb = x^T + b_v); ones at c=256/257 (contiguous memset) give the
softmax-denominator column.

  expS1T[j,i] = exp(T2^T x1 + psi1_j)   (ScalarE, bf16 out, no max-sub)
  outT[i, 0:258] = sum_j expS1T[j,i] * [VT2 | 1 | 1][j, :]
     -> y1 = outT[:,0:256]/outT[:,256] + xtb1
  (symmetric for direction 2)
Chunks of 512 i-columns are software-pipelined: expS(k) matmuls+exps
woven with out(k-1) matmuls; dummy warm-up matmuls keep the PE HAM clock
at 2.4GHz during the input DMA window.
"""

import sys

if "/opt/trn_rl_repo" not in sys.path:
    sys.path.insert(0, "/opt/trn_rl_repo")

import numpy as np

B, C, H, W = 8, 256, 48, 48
N = H * W  # 2304
NT = N // 128  # 18 j/i tiles
CT = C // 128  # 2 c tiles
CW = 512  # max i-chunk width for expS (last chunk is 256)
CHUNKS = [(0, 512), (512, 512), (1024, 512), (1536, 512), (2048, 256)]
# direction 2 ends the kernel: split its last chunk so the un-overlappable
# tail (out-matmuls + epilogue after the final exp) is half as long
CHUNKS2 = [(0, 512), (512, 512), (1024, 512), (1536, 512), (2048, 128), (2176, 128)]

_CACHE = {}


def _build():
    import concourse.bacc as bacc
    import concourse.mybir as mybir
    from concourse.tile import TileContext

    F32, F16, BF16 = mybir.dt.float32, mybir.dt.float16, mybir.dt.bfloat16
    Exp = mybir.ActivationFunctionType.Exp
    Ident = mybir.ActivationFunctionType.Identity

    nc = bacc.Bacc(None, target_bir_lowering=False)

    x_d = {
        "x1": nc.dram_tensor("x1", [C, N], F16, kind="ExternalInput"),
        "x2": nc.dram_tensor("x2", [C, N], F16, kind="ExternalInput"),
    }
    xt_d = {
        "xtb1": nc.dram_tensor("xtb1", [N, C], F32, kind="ExternalInput"),
        "xtb2": nc.dram_tensor("xtb2", [N, C], F32, kind="ExternalInput"),
    }
    w_names = ["g1t", "wv2t", "g2t", "wv1t"]  # pack order
    wpack_d = nc.dram_tensor("wpack", [C, 4 * C], F16, kind="ExternalInput")
    psi_d = nc.dram_tensor("psi", [128, 2 * NT], F32, kind="ExternalInput")
    y_d = {
        "y1t": nc.dram_tensor("y1t", [N, C], F32, kind="ExternalOutput"),
        "y2t": nc.dram_tensor("y2t", [N, C], F32, kind="ExternalOutput"),
    }

    with TileContext(nc) as tc:
        with (
            tc.tile_pool(name="const", bufs=1) as cp,
            tc.tile_pool(name="proj", bufs=1) as pp,
            tc.tile_pool(name="stream", bufs=4) as sp,
            tc.tile_pool(name="psum", bufs=2, space="PSUM") as psp,
            tc.tile_pool(name="psum_s", bufs=3, space="PSUM") as psp2,
        ):
            # ---------- setup: warm-up + input loads ----------
            proj = {}
            # PE warm-up while input DMAs are in flight: keeps HAM at 8/8
            dummy = cp.tile([128, 512], F16, tag="warm")
            nc.vector.memset(dummy[:, :], 0.0)
            wps = None
            for _ in range(16):
                wps = psp.tile([128, 512], F32, tag="ps_o")
                nc.tensor.matmul(
                    wps[:, :], dummy[:, 0:128], dummy[:, :], start=True, stop=True
                )
            wexp = cp.tile([128, 512], F32, tag="warm_exp")
            nc.scalar.activation(wexp[:, :], wps[:, :], Exp)

            x_sb = {}

            def load_x_cols(n, c0, w, npieces):
                # many small transfers spread over the 16 DMA queues: each
                # queue moves ~25GB/s, so parallelism across queues sets the
                # arrival time of the gating tensors
                if n not in x_sb:
                    t = pp.tile([128, CT, N], F16, tag=n, name=n)
                    x_sb[n] = t
                t = x_sb[n]
                pw = w // npieces
                for ck in range(CT):
                    for p in range(npieces):
                        o = c0 + p * pw
                        nc.sync.dma_start(
                            out=t[:, ck, o : o + pw],
                            in_=x_d[n][ck * 128 : (ck + 1) * 128, o : o + pw],
                        )

            wpack = cp.tile([128, CT, 4 * C], F16, tag="wpack")
            w_sb = {n: wpack[:, :, i * C : (i + 1) * C] for i, n in enumerate(w_names)}
            psi_sb = cp.tile([128, 2 * NT], F32, tag="psi")

            def load_w(wn):
                i = w_names.index(wn)
                for ck in range(CT):
                    nc.sync.dma_start(
                        out=wpack[:, ck, i * C : (i + 1) * C],
                        in_=wpack_d[ck * 128 : (ck + 1) * 128, i * C : (i + 1) * C],
                    )

            # DMA order = the critical path of the software pipeline: the
            # T2 = G1 x2 projection (g1t + all of x2) gates dir-1 attention,
            # then chunk-0 logits need the first half of x1, the first exps
            # need psi, and the VT2 fill (woven into chunk 0) needs wv2t.
            load_w("g1t")
            load_x_cols("x2", 0, N, 6)
            load_x_cols("x1", 0, N // 2, 4)
            nc.sync.dma_start(out=psi_sb[:, :], in_=psi_d[:, :])
            load_w("wv2t")
            load_x_cols("x1", N // 2, N // 2, 4)
            load_w("g2t")
            load_w("wv1t")

            # ---------- projection action builders ----------
            # T = G x : out[c', n] = sum_c Gt[c, c'] x[c, n]; no bias.
            def proj_t_actions(dst, xt, wn, alt0=0):
                # chunk-major so early actions only need the first half of xt
                acts = []
                i = 0
                for c0, cw in CHUNKS:
                    for ct in range(CT):

                        def mk(ct, c0, cw, use_act):
                            def act():
                                ps2 = psp2.tile([128, 2, CW], F32, tag="ps_s")
                                ps = ps2[:, 0, :]
                                for ck in range(CT):
                                    nc.tensor.matmul(
                                        ps[:, 0:cw],
                                        w_sb[wn][:, ck, ct * 128 : (ct + 1) * 128],
                                        xt[:, ck, c0 : c0 + cw],
                                        start=(ck == 0),
                                        stop=(ck == CT - 1),
                                    )
                                if use_act:
                                    nc.scalar.activation(
                                        dst[:, ct, c0 : c0 + cw], ps[:, 0:cw], Ident
                                    )
                                else:
                                    nc.vector.tensor_copy(
                                        dst[:, ct, c0 : c0 + cw], ps[:, 0:cw]
                                    )

                            return act

                        acts.append(mk(ct, c0, cw, (alt0 + i) % 2 == 0))
                        i += 1
                return acts

            # V: outT[j, c] = sum_cin x[cin, j] w[cin, c]; no bias (folded
            # into xtb on host); cols 256/257 are ones (memset once).
            def proj_vt_actions(dst, xt, wn):
                acts = []
                for jt in range(NT):

                    def mk(jt):
                        def act():
                            ps2 = psp2.tile([128, 2, CW], F32, tag="ps_s")
                            ps = ps2[:, 0, :]
                            for ck in range(CT):
                                nc.tensor.matmul(
                                    ps[:, 0:C],
                                    xt[:, ck, jt * 128 : (jt + 1) * 128],
                                    w_sb[wn][:, ck, :],
                                    start=(ck == 0),
                                    stop=(ck == CT - 1),
                                )
                            if jt % 2 == 0:
                                nc.vector.tensor_copy(dst[:, jt, 0:C], ps[:, 0:C])
                            else:
                                nc.scalar.activation(dst[:, jt, 0:C], ps[:, 0:C], Ident)

                        return act

                    acts.append(mk(jt))
                return acts

            for nm in ["T2", "T1"]:
                proj[nm] = pp.tile([128, CT, N], F16, tag=nm, name=nm)
            for nm in ["VT2", "VT1"]:
                proj[nm] = pp.tile([128, NT, C + 2], BF16, tag=nm, name=nm)

            # only T2 must precede dir-1 attention; VT2 is consumed by
            # out(c0) whose emission starts in chunk 1, so VT2 and all dir-2
            # projections become fill work woven into dir-1's attention chunks
            for a in proj_t_actions(proj["T2"], x_sb["x2"], "g1t", 0):
                a()
            for nm in ["VT2", "VT1"]:
                # ones-fill the whole tile (contiguous memset, AFTER the T2
                # emission so the DVE queue reaches T2's PSUM copies first);
                # the V writes overwrite cols 0:256 of each j-tile, leaving
                # ones at c=256/257 as the softmax-denominator columns
                nc.vector.memset(proj[nm][:, :, :], 1.0)
            vt2_acts = proj_vt_actions(proj["VT2"], x_sb["x2"], "wv2t")
            fill = (
                vt2_acts
                + proj_t_actions(proj["T1"], x_sb["x1"], "g2t", 1)
                + proj_vt_actions(proj["VT1"], x_sb["x1"], "wv1t")
            )
            # per-chunk fill quotas: ALL of VT2 must be emitted within chunk 0
            n_vt2 = len(vt2_acts)
            rest = len(fill) - n_vt2 - 4
            quotas = [n_vt2 + 4] + [(rest + 3) // 4] * 4

            # ---------- attention ----------
            with tc.tile_pool(name="ep", bufs=2) as ep:

                def exp_actions(Q, K, e, p0, c0, cw):
                    # one action = expS matmuls for a PAIR of j-tiles + two
                    # exps (per-jt psi bias on the j partitions)
                    def mk(jp):
                        def act():
                            ps2 = psp2.tile([128, 2, CW], F32, tag="ps_s")
                            for jj in range(2):
                                jt = jp + jj
                                for ck in range(CT):
                                    nc.tensor.matmul(
                                        ps2[:, jj, 0:cw],
                                        K[:, ck, jt * 128 : (jt + 1) * 128],
                                        Q[:, ck, c0 : c0 + cw],
                                        start=(ck == 0),
                                        stop=(ck == CT - 1),
                                    )
                            for jj in range(2):
                                jt = jp + jj
                                nc.scalar.activation(
                                    e[:, jt, 0:cw],
                                    ps2[:, jj, 0:cw],
                                    Exp,
                                    bias=psi_sb[:, p0 + jt : p0 + jt + 1],
                                )

                        return act

                    return [mk(jp) for jp in range(0, NT, 2)]

                def out_actions(e, VT, xt_dram, yt_dram, c0, cw):
                    # actions = out-matmul slices + epilogue, per i-subtile
                    acts = []
                    for il in range(cw // 128):
                        it = c0 // 128 + il
                        po = psp.tile([128, C + 2], F32, tag="ps_o")

                        xt_t = sp.tile([128, C], F32, tag="xt")

                        def mk_mm(po, il, it, j0, jn, xt_t):
                            def act():
                                if j0 == 0:
                                    nc.sync.dma_start(
                                        out=xt_t[:, :],
                                        in_=xt_dram[it * 128 : (it + 1) * 128, :],
                                    )
                                for jt in range(j0, jn):
                                    nc.tensor.matmul(
                                        po[:, :],
                                        e[:, jt, il * 128 : (il + 1) * 128],
                                        VT[:, jt, :],
                                        start=(jt == 0),
                                        stop=(jt == NT - 1),
                                    )

                            return act

                        for j0 in range(0, NT, 5):
                            acts.append(mk_mm(po, il, it, j0, min(j0 + 5, NT), xt_t))

                        def mk_epi(po, it, xt_t):
                            def act():
                                r = sp.tile([128, 1], F32, tag="r")
                                nc.vector.reciprocal(r[:, :], po[:, C : C + 1])
                                y = sp.tile([128, C], F32, tag="y")
                                nc.vector.scalar_tensor_tensor(
                                    y[:, :],
                                    po[:, 0:C],
                                    r[:, :],
                                    xt_t[:, :],
                                    op0=mybir.AluOpType.mult,
                                    op1=mybir.AluOpType.add,
                                )
                                nc.sync.dma_start(
                                    out=yt_dram[it * 128 : (it + 1) * 128, :], in_=y[:, :]
                                )

                            return act

                        acts.append(mk_epi(po, it, xt_t))
                    return acts

                def weave(a, b):
                    # emit all of a and b interleaved evenly (a paces, b fills)
                    if not b:
                        for f in a:
                            f()
                        return
                    na, nb = len(a), len(b)
                    j = 0
                    for i, f in enumerate(a):
                        f()
                        while j < nb and j * na <= (i + 1) * nb - 1:
                            b[j]()
                            j += 1
                    while j < nb:
                        b[j]()
                        j += 1

                # software pipeline: expS(k) woven with out(k-1); dir-2
                # projections are fill distributed across dir-1's chunks
                plan = [
                    (x_sb["x1"], proj["T2"], proj["VT2"], 0, xt_d["xtb1"], y_d["y1t"], c0, cw)
                    for c0, cw in CHUNKS
                ] + [
                    (x_sb["x2"], proj["T1"], proj["VT1"], NT, xt_d["xtb2"], y_d["y2t"], c0, cw)
                    for c0, cw in CHUNKS2
                ]
                nd1 = len(CHUNKS)
                pending = []
                for step, (Q, K, VT, p0, xtd, ytd, c0, cw) in enumerate(plan):
                    if step < nd1:
                        q = quotas[step]
                        extra, fill = fill[:q], fill[q:]
                    else:
                        assert not fill
                        extra = []
                    e = ep.tile([128, NT, CW], BF16, tag="e")
                    weave(exp_actions(Q, K, e, p0, c0, cw), pending + extra)
                    pending = out_actions(e, VT, xtd, ytd, c0, cw)
                weave(pending, [])

    nc.compile()
    return nc


def _get_nc():
    if "nc" not in _CACHE:
        _CACHE["nc"] = _build()
    return _CACHE["nc"]


def kernel(
    x1,
    x2,
    w_q1,
    b_q1,
    w_k1,
    b_k1,
    w_v1,
    b_v1,
    w_q2,
    b_q2,
    w_k2,
    b_k2,
    w_v2,
    b_v2,
    _trace=False,
):
    from concourse.bass_utils import run_bass_kernel_spmd

    nc = _get_nc()

    x1 = np.asarray(x1, dtype=np.float32)
    x2 = np.asarray(x2, dtype=np.float32)
    x1h = x1.astype(np.float16)
    x2h = x2.astype(np.float16)
    wq1 = np.asarray(w_q1, np.float32)
    wk1 = np.asarray(w_k1, np.float32)
    wq2 = np.asarray(w_q2, np.float32)
    wk2 = np.asarray(w_k2, np.float32)
    # G1 = Wq1^T Wk2, G2 = Wq2^T Wk1; pack transposed (g1t = G1^T) to match
    # the lhsT convention. Order: g1t, wv2t, g2t, wv1t.
    g1t = (wq1.T @ wk2).T  # = Wk2^T Wq1
    g2t = (wq2.T @ wk1).T
    wpack = np.ascontiguousarray(
        np.concatenate(
            [g1t, np.asarray(w_v2, np.float32).T, g2t, np.asarray(w_v1, np.float32).T],
            axis=1,
        ).astype(np.float16)
    )
    bq1 = np.asarray(b_q1, np.float32)
    bk1 = np.asarray(b_k1, np.float32)
    bq2 = np.asarray(b_q2, np.float32)
    bk2 = np.asarray(b_k2, np.float32)
    bv1 = np.asarray(b_v1, np.float32)
    bv2 = np.asarray(b_v2, np.float32)
    # per-j logit shifts: psi1_j = bq1.(Wk2 x2_j + bk2); the per-i term
    # (Q1_i.bk2) is softmax-invariant and dropped.
    g1v = wk2.T @ bq1
    c1 = float(bq1 @ bk2)
    g2v = wk1.T @ bq2
    c2 = float(bq2 @ bk1)

    in_maps = []
    for i in range(B):
        x1i = np.ascontiguousarray(x1[i].reshape(C, N))
        x2i = np.ascontiguousarray(x2[i].reshape(C, N))
        psi1 = g1v @ x2i + c1  # [N]
        psi2 = g2v @ x1i + c2
        psi = np.concatenate(
            [psi1.reshape(NT, 128).T, psi2.reshape(NT, 128).T], axis=1
        ).astype(np.float32)
        m = {
            "x1": np.ascontiguousarray(x1h[i].reshape(C, N)),
            "x2": np.ascontiguousarray(x2h[i].reshape(C, N)),
            # residual streams with the V-bias of the opposite direction
            # folded in: y1 = A1.(V2+bv2) + x1 = A1.V2/den + (x1^T + bv2)^T
            "xtb1": np.ascontiguousarray(x1i.T + bv2[None, :]),
            "xtb2": np.ascontiguousarray(x2i.T + bv1[None, :]),
            "wpack": wpack,
            "psi": np.ascontiguousarray(psi),
        }
        in_maps.append(m)

    res = run_bass_kernel_spmd(nc, in_maps, list(range(B)), trace=_trace)
    if _trace:
        _CACHE["last_result"] = res

    y1 = np.empty((B, C, H, W), np.float32)
    y2 = np.empty((B, C, H, W), np.float32)
    for i in range(B):
        y1[i] = res.results[i]["y1t"].T.reshape(C, H, W)
        y2[i] = res.results[i]["y2t"].T.reshape(C, H, W)
    return y1, y2


# revision 22
# speedup vs baseline: 1.0167x; 1.0167x over previous
"""Bidirectional cross-attention Trainium2 kernel (G-folded projections).

Data-parallel over batch B=8 across 8 NeuronCores (1 sample/core).

Logits are computed via the associativity fold
  S1 = Q1^T K2 = x1^T (Wq1^T Wk2) x2 = x1^T T2,   T2 = G1 x2
with G1 = Wq1^T Wk2 precomputed on host — this removes BOTH the Q and K
projections (one T-projection remains per direction) and lets raw x1 act
as the moving operand of the logit matmuls. The bias cross-terms
decompose into a per-i shift (cancels in softmax -> dropped exactly) and
a per-j shift psi_j = bq1.(Wk2 x2_j + bk2), computed on host (O(CN)) and
applied for free through the per-partition bias port of the ScalarE exp.

V path: VT[n,c] bf16 with NO bias (b_v commutes through softmax since
weights sum to 1 after the division; host folds it into the residual
stream xtb = x^T + b_v); ones at c=256/257 (contiguous memset) give the
softmax-denominator column.

  expS1T[j,i] = exp(T2^T x1 + psi1_j)   (ScalarE, bf16 out, no max-sub)
  outT[i, 0:258] = sum_j expS1T[j,i] * [VT2 | 1 | 1][j, :]
     -> y1 = outT[:,0:256]/outT[:,256] + xtb1
  (symmetric for direction 2)
Chunks of 512 i-columns are software-pipelined: expS(k) matmuls+exps
woven with out(k-1) matmuls; dummy warm-up matmuls keep the PE HAM clock
at 2.4GHz during the input DMA window.
"""

import sys

if "/opt/trn_rl_repo" not in sys.path:
    sys.path.insert(0, "/opt/trn_rl_repo")

import numpy as np

B, C, H, W = 8, 256, 48, 48
N = H * W  # 2304
NT = N // 128  # 18 j/i tiles
CT = C // 128  # 2 c tiles
CW = 512  # max i-chunk width for expS (last chunk is 256)
CHUNKS = [(0, 512), (512, 512), (1024, 512), (1536, 512), (2048, 256)]
# direction 2 ends the kernel: split its last chunk so the un-overlappable
# tail (out-matmuls + epilogue after the final exp) is half as long
CHUNKS2 = [(0, 512), (512, 512), (1024, 512), (1536, 512), (2048, 128), (2176, 128)]

_CACHE = {}


def _build():
    import concourse.bacc as bacc
    import concourse.mybir as mybir
    from concourse.tile import TileContext

    F32, F16, BF16 = mybir.dt.float32, mybir.dt.float16, mybir.dt.bfloat16
    Exp = mybir.ActivationFunctionType.Exp
    Ident = mybir.ActivationFunctionType.Identity

    nc = bacc.Bacc(None, target_bir_lowering=False)

    x_d = {
        "x1": nc.dram_tensor("x1", [C, N], F16, kind="ExternalInput"),
        "x2": nc.dram_tensor("x2", [C, N], F16, kind="ExternalInput"),
    }
    xt_d = {
        "xtb1": nc.dram_tensor("xtb1", [N, C], F32, kind="ExternalInput"),
        "xtb2": nc.dram_tensor("xtb2", [N, C], F32, kind="ExternalInput"),
    }
    w_names = ["g1t", "wv2t", "g2t", "wv1t"]  # pack order
    wpack_d = nc.dram_tensor("wpack", [C, 4 * C], F16, kind="ExternalInput")
    psi_d = nc.dram_tensor("psi", [128, 2 * NT], F32, kind="ExternalInput")
    y_d = {
        "y1t": nc.dram_tensor("y1t", [N, C], F32, kind="ExternalOutput"),
        "y2t": nc.dram_tensor("y2t", [N, C], F32, kind="ExternalOutput"),
    }

    with TileContext(nc) as tc:
        with (
            tc.tile_pool(name="const", bufs=1) as cp,
            tc.tile_pool(name="proj", bufs=1) as pp,
            tc.tile_pool(name="stream", bufs=4) as sp,
            tc.tile_pool(name="psum", bufs=2, space="PSUM") as psp,
            tc.tile_pool(name="psum_s", bufs=3, space="PSUM") as psp2,
        ):
            # ---------- setup: warm-up + input loads ----------
            proj = {}
            # PE warm-up while input DMAs are in flight: keeps HAM at 8/8
            dummy = cp.tile([128, 512], F16, tag="warm")
            nc.vector.memset(dummy[:, :], 0.0)
            wps = None
            for _ in range(25):
                wps = psp.tile([128, 512], F32, tag="ps_o")
                nc.tensor.matmul(
                    wps[:, :], dummy[:, 0:128], dummy[:, :], start=True, stop=True
                )
            wexp = cp.tile([128, 512], F32, tag="warm_exp")
            nc.scalar.activation(wexp[:, :], wps[:, :], Exp)

            x_sb = {}

            def load_x_half(n, h0):
                if n not in x_sb:
                    t = pp.tile([128, CT, N], F16, tag=n, name=n)
                    x_sb[n] = t
                t = x_sb[n]
                for ck in range(CT):
                    nc.sync.dma_start(
                        out=t[:, ck, h0 : h0 + N // 2],
                        in_=x_d[n][ck * 128 : (ck + 1) * 128, h0 : h0 + N // 2],
                    )

            wpack = cp.tile([128, CT, 4 * C], F16, tag="wpack")
            w_sb = {n: wpack[:, :, i * C : (i + 1) * C] for i, n in enumerate(w_names)}
            psi_sb = cp.tile([128, 2 * NT], F32, tag="psi")

            def load_w(wn):
                i = w_names.index(wn)
                for ck in range(CT):
                    nc.sync.dma_start(
                        out=wpack[:, ck, i * C : (i + 1) * C],
                        in_=wpack_d[ck * 128 : (ck + 1) * 128, i * C : (i + 1) * C],
                    )

            # DMA order = the critical path of the software pipeline: the
            # T2 = G1 x2 projection (g1t + all of x2) gates dir-1 attention,
            # then chunk-0 logits need the first half of x1, the first exps
            # need psi, and the VT2 fill (woven into chunk 0) needs wv2t.
            load_w("g1t")
            load_x_half("x2", 0)
            load_x_half("x2", N // 2)
            load_x_half("x1", 0)
            nc.sync.dma_start(out=psi_sb[:, :], in_=psi_d[:, :])
            load_w("wv2t")
            load_x_half("x1", N // 2)
            load_w("g2t")
            load_w("wv1t")

            # ---------- projection action builders ----------
            # T = G x : out[c', n] = sum_c Gt[c, c'] x[c, n]; no bias.
            def proj_t_actions(dst, xt, wn, alt0=0):
                # chunk-major so early actions only need the first half of xt
                acts = []
                i = 0
                for c0, cw in CHUNKS:
                    for ct in range(CT):

                        def mk(ct, c0, cw, use_act):
                            def act():
                                ps2 = psp2.tile([128, 2, CW], F32, tag="ps_s")
                                ps = ps2[:, 0, :]
                                for ck in range(CT):
                                    nc.tensor.matmul(
                                        ps[:, 0:cw],
                                        w_sb[wn][:, ck, ct * 128 : (ct + 1) * 128],
                                        xt[:, ck, c0 : c0 + cw],
                                        start=(ck == 0),
                                        stop=(ck == CT - 1),
                                    )
                                if use_act:
                                    nc.scalar.activation(
                                        dst[:, ct, c0 : c0 + cw], ps[:, 0:cw], Ident
                                    )
                                else:
                                    nc.vector.tensor_copy(
                                        dst[:, ct, c0 : c0 + cw], ps[:, 0:cw]
                                    )

                            return act

                        acts.append(mk(ct, c0, cw, (alt0 + i) % 2 == 0))
                        i += 1
                return acts

            # V: outT[j, c] = sum_cin x[cin, j] w[cin, c]; no bias (folded
            # into xtb on host); cols 256/257 are ones (memset once).
            def proj_vt_actions(dst, xt, wn):
                acts = []
                for jt in range(NT):

                    def mk(jt):
                        def act():
                            ps2 = psp2.tile([128, 2, CW], F32, tag="ps_s")
                            ps = ps2[:, 0, :]
                            for ck in range(CT):
                                nc.tensor.matmul(
                                    ps[:, 0:C],
                                    xt[:, ck, jt * 128 : (jt + 1) * 128],
                                    w_sb[wn][:, ck, :],
                                    start=(ck == 0),
                                    stop=(ck == CT - 1),
                                )
                            if jt % 2 == 0:
                                nc.vector.tensor_copy(dst[:, jt, 0:C], ps[:, 0:C])
                            else:
                                nc.scalar.activation(dst[:, jt, 0:C], ps[:, 0:C], Ident)

                        return act

                    acts.append(mk(jt))
                return acts

            for nm in ["T2", "T1"]:
                proj[nm] = pp.tile([128, CT, N], F16, tag=nm, name=nm)
            for nm in ["VT2", "VT1"]:
                proj[nm] = pp.tile([128, NT, C + 2], BF16, tag=nm, name=nm)

            # only T2 must precede dir-1 attention; VT2 is consumed by
            # out(c0) whose emission starts in chunk 1, so VT2 and all dir-2
            # projections become fill work woven into dir-1's attention chunks
            for a in proj_t_actions(proj["T2"], x_sb["x2"], "g1t", 0):
                a()
            for nm in ["VT2", "VT1"]:
                # ones-fill the whole tile (contiguous memset, AFTER the T2
                # emission so the DVE queue reaches T2's PSUM copies first);
                # the V writes overwrite cols 0:256 of each j-tile, leaving
                # ones at c=256/257 as the softmax-denominator columns
                nc.vector.memset(proj[nm][:, :, :], 1.0)
            vt2_acts = proj_vt_actions(proj["VT2"], x_sb["x2"], "wv2t")
            fill = (
                vt2_acts
                + proj_t_actions(proj["T1"], x_sb["x1"], "g2t", 1)
                + proj_vt_actions(proj["VT1"], x_sb["x1"], "wv1t")
            )
            # per-chunk fill quotas: ALL of VT2 must be emitted within chunk 0
            n_vt2 = len(vt2_acts)
            rest = len(fill) - n_vt2 - 4
            quotas = [n_vt2 + 4] + [(rest + 3) // 4] * 4

            # ---------- attention ----------
            with tc.tile_pool(name="ep", bufs=2) as ep:

                def exp_actions(Q, K, e, p0, c0, cw):
                    # one action = expS matmuls for a PAIR of j-tiles + two
                    # exps (per-jt psi bias on the j partitions)
                    def mk(jp):
                        def act():
                            ps2 = psp2.tile([128, 2, CW], F32, tag="ps_s")
                            for jj in range(2):
                                jt = jp + jj
                                for ck in range(CT):
                                    nc.tensor.matmul(
                                        ps2[:, jj, 0:cw],
                                        K[:, ck, jt * 128 : (jt + 1) * 128],
                                        Q[:, ck, c0 : c0 + cw],
                                        start=(ck == 0),
                                        stop=(ck == CT - 1),
                                    )
                            for jj in range(2):
                                jt = jp + jj
                                nc.scalar.activation(
                                    e[:, jt, 0:cw],
                                    ps2[:, jj, 0:cw],
                                    Exp,
                                    bias=psi_sb[:, p0 + jt : p0 + jt + 1],
                                )

                        return act

                    return [mk(jp) for jp in range(0, NT, 2)]

                def out_actions(e, VT, xt_dram, yt_dram, c0, cw):
                    # actions = out-matmul slices + epilogue, per i-subtile
                    acts = []
                    for il in range(cw // 128):
                        it = c0 // 128 + il
                        po = psp.tile([128, C + 2], F32, tag="ps_o")

                        xt_t = sp.tile([128, C], F32, tag="xt")

                        def mk_mm(po, il, it, j0, jn, xt_t):
                            def act():
                                if j0 == 0:
                                    nc.sync.dma_start(
                                        out=xt_t[:, :],
                                        in_=xt_dram[it * 128 : (it + 1) * 128, :],
                                    )
                                for jt in range(j0, jn):
                                    nc.tensor.matmul(
                                        po[:, :],
                                        e[:, jt, il * 128 : (il + 1) * 128],
                                        VT[:, jt, :],
                                        start=(jt == 0),
                                        stop=(jt == NT - 1),
                                    )

                            return act

                        for j0 in range(0, NT, 5):
                            acts.append(mk_mm(po, il, it, j0, min(j0 + 5, NT), xt_t))

                        def mk_epi(po, it, xt_t):
                            def act():
                                r = sp.tile([128, 1], F32, tag="r")
                                nc.vector.reciprocal(r[:, :], po[:, C : C + 1])
                                y = sp.tile([128, C], F32, tag="y")
                                nc.vector.scalar_tensor_tensor(
                                    y[:, :],
                                    po[:, 0:C],
                                    r[:, :],
                                    xt_t[:, :],
                                    op0=mybir.AluOpType.mult,
                                    op1=mybir.AluOpType.add,
                                )
                                nc.sync.dma_start(
                                    out=yt_dram[it * 128 : (it + 1) * 128, :], in_=y[:, :]
                                )

                            return act

                        acts.append(mk_epi(po, it, xt_t))
                    return acts

                def weave(a, b):
                    # emit all of a and b interleaved evenly (a paces, b fills)
                    if not b:
                        for f in a:
                            f()
                        return
                    na, nb = len(a), len(b)
                    j = 0
                    for i, f in enumerate(a):
                        f()
                        while j < nb and j * na <= (i + 1) * nb - 1:
                            b[j]()
                            j += 1
                    while j < nb:
                        b[j]()
                        j += 1

                # software pipeline: expS(k) woven with out(k-1); dir-2
                # projections are fill distributed across dir-1's chunks
                plan = [
                    (x_sb["x1"], proj["T2"], proj["VT2"], 0, xt_d["xtb1"], y_d["y1t"], c0, cw)
                    for c0, cw in CHUNKS
                ] + [
                    (x_sb["x2"], proj["T1"], proj["VT1"], NT, xt_d["xtb2"], y_d["y2t"], c0, cw)
                    for c0, cw in CHUNKS2
                ]
                nd1 = len(CHUNKS)
                pending = []
                for step, (Q, K, VT, p0, xtd, ytd, c0, cw) in enumerate(plan):
                    if step < nd1:
                        q = quotas[step]
                        extra, fill = fill[:q], fill[q:]
                    else:
                        assert not fill
                        extra = []
                    e = ep.tile([128, NT, CW], BF16, tag="e")
                    weave(exp_actions(Q, K, e, p0, c0, cw), pending + extra)
                    pending = out_actions(e, VT, xtd, ytd, c0, cw)
                weave(pending, [])

    nc.compile()
    return nc


def _get_nc():
    if "nc" not in _CACHE:
        _CACHE["nc"] = _build()
    return _CACHE["nc"]


def kernel(
    x1,
    x2,
    w_q1,
    b_q1,
    w_k1,
    b_k1,
    w_v1,
    b_v1,
    w_q2,
    b_q2,
    w_k2,
    b_k2,
    w_v2,
    b_v2,
    _trace=False,
):
    from concourse.bass_utils import run_bass_kernel_spmd

    nc = _get_nc()

    x1 = np.asarray(x1, dtype=np.float32)
    x2 = np.asarray(x2, dtype=np.float32)
    x1h = x1.astype(np.float16)
    x2h = x2.astype(np.float16)
    wq1 = np.asarray(w_q1, np.float32)
    wk1 = np.asarray(w_k1, np.float32)
    wq2 = np.asarray(w_q2, np.float32)
    wk2 = np.asarray(w_k2, np.float32)
    # G1 = Wq1^T Wk2, G2 = Wq2^T Wk1; pack transposed (g1t = G1^T) to match
    # the lhsT convention. Order: g1t, wv2t, g2t, wv1t.
    g1t = (wq1.T @ wk2).T  # = Wk2^T Wq1
    g2t = (wq2.T @ wk1).T
    wpack = np.ascontiguousarray(
        np.concatenate(
            [g1t, np.asarray(w_v2, np.float32).T, g2t, np.asarray(w_v1, np.float32).T],
            axis=1,
        ).astype(np.float16)
    )
    bq1 = np.asarray(b_q1, np.float32)
    bk1 = np.asarray(b_k1, np.float32)
    bq2 = np.asarray(b_q2, np.float32)
    bk2 = np.asarray(b_k2, np.float32)
    bv1 = np.asarray(b_v1, np.float32)
    bv2 = np.asarray(b_v2, np.float32)
    # per-j logit shifts: psi1_j = bq1.(Wk2 x2_j + bk2); the per-i term
    # (Q1_i.bk2) is softmax-invariant and dropped.
    g1v = wk2.T @ bq1
    c1 = float(bq1 @ bk2)
    g2v = wk1.T @ bq2
    c2 = float(bq2 @ bk1)

    in_maps = []
    for i in range(B):
        x1i = np.ascontiguousarray(x1[i].reshape(C, N))
        x2i = np.ascontiguousarray(x2[i].reshape(C, N))
        psi1 = g1v @ x2i + c1  # [N]
        psi2 = g2v @ x1i + c2
        psi = np.concatenate(
            [psi1.reshape(NT, 128).T, psi2.reshape(NT, 128).T], axis=1
        ).astype(np.float32)
        m = {
            "x1": np.ascontiguousarray(x1h[i].reshape(C, N)),
            "x2": np.ascontiguousarray(x2h[i].reshape(C, N)),
            # residual streams with the V-bias of the opposite direction
            # folded in: y1 = A1.(V2+bv2) + x1 = A1.V2/den + (x1^T + bv2)^T
            "xtb1": np.ascontiguousarray(x1i.T + bv2[None, :]),
            "xtb2": np.ascontiguousarray(x2i.T + bv1[None, :]),
            "wpack": wpack,
            "psi": np.ascontiguousarray(psi),
        }
        in_maps.append(m)

    res = run_bass_kernel_spmd(nc, in_maps, list(range(B)), trace=_trace)
    if _trace:
        _CACHE["last_result"] = res

    y1 = np.empty((B, C, H, W), np.float32)
    y2 = np.empty((B, C, H, W), np.float32)
    for i in range(B):
        y1[i] = res.results[i]["y1t"].T.reshape(C, H, W)
        y2[i] = res.results[i]["y2t"].T.reshape(C, H, W)
    return y1, y2
